# revision 27
# baseline (speedup 1.0000x reference)
"""CrossAttention3D Trainium2 kernel.

Full inputs in, full output out. Sharding: data-parallel over batch (2) x
query-token shards (4) = 8 NeuronCores; each core runs 1024 queries against
all 4096 keys.

Host-side algebraic folding removes ALL four projections from the device:
  scores = src^T (Wq^T Wk / sqrt(C)) tgt: Q' = (M^T src) is O(N C^2) and is
  computed on the host (shipped as fp16), so the device's score matmuls use
  Q' (moving) against raw tgt tiles (stationary).
  V' = (wo wv) tgt is likewise host-computed and shipped pre-transposed as
  [key, kt, c] tiles with a ones column appended - the PV contraction then
  produces output channels AND the softmax denominator in one accumulation
  (wo@bv + bo added on the host at the end).

Scores are produced pre-scaled by SC = 1024/ln2. exp runs split across two
engines per key tile (alternating):
  Act: et = Exp(st * ln2/1024 + beta)        (exact fp16 exp)
  DVE: et = bits_f16(i16(max(st,-CL) + MAGIC+beta*SC))   (Schraudolph-style
       exponent-field trick: one tensor_scalar op; ~3% max elementwise err,
       ~5e-3 end-to-end because softmax numerator/denominator errors cancel)
The per-key bias beta = (bq.Wk tgt_k + bq.bk)/sqrt(C) enters the Act bias /
DVE add operand (zero for this problem's inputs but kept for generality).

The epilogue ships the raw accumulated [q, c | denom] PSUM regions to DRAM
as fp32; the host performs the final normalize/transpose. This removes all
post-PV device work so the tail is just three PSUM->DRAM DMAs.
"""

import numpy as np

import concourse.bass as bass
import concourse.mybir as mybir
import concourse.tile as tile
from concourse.bass_utils import run_bass_kernel_spmd
from concourse.vector_clock import ScopedClock

F32 = mybir.dt.float32
F16 = mybir.dt.float16
I16 = mybir.dt.int16

B, C, D, H, W = 2, 128, 16, 16, 16
N = D * H * W          # 4096 target tokens
NCORES = 8
QSHARDS = NCORES // B  # 4 query shards per batch
NQ = N // QSHARDS      # 1024 query tokens per core
KT = N // 128          # 32 key tiles
QC = NQ // 128         # 8 query chunks of 128
AF = mybir.ActivationFunctionType
OP = mybir.AluOpType

SC = 1024.0 / np.log(2.0)          # scores pre-scale: exp(s) = 2^(s*SC/1024)
LN2_1024 = float(np.log(2.0) / 1024.0)
MAGIC = 15360.0 - 45.0             # f16 exponent-field magic (RMS-tuned)
CLAMP = -(MAGIC - 3.0)             # keep i16 value >= 3 (graceful denormals)


# --- walrus sync-wait workarounds (unchanged from baseline) ------------------

def _patched_drain_and_barrier(self, tick_clock, wait_clock):
    # This walrus build caps sync-waits per instruction; the stock TileContext
    # exit drain carries one wait per processor lane (>4 in this kernel).
    # Split the waits into single-wait SP instructions before the drain.
    nc = self.nc
    probe = nc.sync.nop()
    wait_clock.add_sem_waits(probe.ins, ScopedClock({None: tick_clock.global_clock}))
    si = probe.ins.sync_info
    waits = list(si.on_wait) if si and si.on_wait else []
    if si:
        si.on_wait = []
        probe.ins.sync_info = si
    by_name = {h.name: h for h in self.sems.allocated().values()}
    opmap = {"sem-ge-imm": "sem-ge", "sem-eq-imm": "sem-eq"}
    for wv in waits:
        nc.sync.wait_op(by_name[wv.ant_name], wv.wait_value, opmap.get(wv.wait_mode, "sem-ge"))
    nc.sync.drain()
    nc.all_engine_barrier()
    popped = nc._tile_sem_poison_stack.pop()
    assert popped is self._sem_poison
    nc.clear_and_free_semaphores(list(self.sems.allocated().values()))


tile.TileContext._drain_and_barrier = _patched_drain_and_barrier


def _split_excess_waits(nc, cap=1, evsem_cap=2):
    # This walrus build rejects instructions carrying more than ~1 sync wait
    # (Tile targets a newer walrus that packs several). Hoist excess waits
    # onto dedicated InstEventSemaphore instructions just before the
    # over-subscribed instruction, on the same engine stream.
    for fn in nc.m.functions:
        for bb in fn.blocks:
            out = []
            for inst in bb.instructions:
                si = inst.sync_info
                waits = list(si.on_wait) if si and si.on_wait else []
                limit = (
                    evsem_cap
                    if isinstance(inst, (mybir.InstEventSemaphore, mybir.InstDrain))
                    else cap
                )
                if len(waits) > limit:
                    excess, keep = waits[:-limit], waits[-limit:]
                    for i in range(0, len(excess), evsem_cap):
                        ev = mybir.InstEventSemaphore(
                            name=nc.get_next_instruction_name(),
                            engine=inst.engine,
                            ins=[],
                            outs=[],
                            sync_info=mybir.SyncInfo(
                                on_wait=excess[i : i + evsem_cap], on_update=[]
                            ),
                        )
                        nc.register_instruction(ev)
                        out.append(ev)
                    si.on_wait = keep
                    inst.sync_info = si
                out.append(inst)
            bb.instructions[:] = out


# --- kernel ------------------------------------------------------------------

def build_bass():
    nc = bass.Bass("TRN2", target_bir_lowering=False, debug=False)

    # single merged f16 input: [ Q'*SC (1024) | tgt (4096) | V'^T+ones (32*129) ]
    NB = NQ + N + KT * (C + 1)
    VOFF = NQ + N
    buf = nc.dram_tensor("buf", [C, NB], F16, kind="ExternalInput")
    bets = nc.dram_tensor("bets", [C, 2, KT], F32, kind="ExternalInput")
    # raw accumulations [q_part, chunk, c|denom] per psum region
    out = nc.dram_tensor("out", [C, QC, C + 1], F16, kind="ExternalOutput")

    # opsum chunk j -> (psum tile, region index). Three 129-wide fp32 regions
    # at 170-float stride fit one 2KB bank; start=True only on region 0 (the
    # bank-wide has_written clear opens the bank-mates' groups too).
    CHUNK_MAP = [(0, 0), (0, 1), (0, 2), (1, 0), (1, 1), (1, 2), (2, 0), (2, 1)]

    with tile.TileContext(nc) as tc:
        with (
            tc.tile_pool(name="consts", bufs=1) as consts,
            tc.tile_pool(name="big", bufs=1) as big,
            tc.tile_pool(name="ets", bufs=4) as ets,
            tc.tile_pool(name="psum_st", bufs=5, space="PSUM") as psum_st,
            tc.tile_pool(name="psum_pv", bufs=1, space="PSUM") as psum_pv,
        ):
            # Warm-up matmuls read a framework-materialized const AP (no
            # engine init op of our own), so they start as early as possible
            # and ramp the PE clock.
            BF16 = mybir.dt.bfloat16
            warm_w = nc.const_aps.tensor(1.0, (C, 1), BF16)
            warm_m = nc.const_aps.tensor(1.0, (C, 512), BF16)

            # ---- inputs. The HWDGE descriptor generator is a single global
            # resource and transfers queue in gen order, so issue everything
            # on one ring, sliced strictly by need-time.
            buf_sb = big.tile([C, NB], F16)
            bets_sb = consts.tile([C, 2, KT], F32)

            def qp_ap(lo, hi):
                return buf_sb[:, lo:hi]

            def tgt_ap(kt):
                return buf_sb[:, NQ + kt * 128 : NQ + (kt + 1) * 128]

            def vta_ap(kt):
                return buf_sb[:, VOFF + kt * (C + 1) : VOFF + (kt + 1) * (C + 1)]

            def _in(lo, hi):
                nc.sync.dma_start(buf_sb[:, lo:hi], buf[:, lo:hi])

            _in(0, 1280)                       # qp + tgt tiles 0-1
            nc.sync.dma_start(bets_sb[:], bets[:, :, :])
            _in(1280, 1536)                    # tgt tiles 2-3
            _in(VOFF, VOFF + 2 * (C + 1))      # vta tiles 0-1
            _in(1536, 2304)                    # tgt tiles 4-9
            _in(VOFF + 2 * (C + 1), VOFF + 8 * (C + 1))   # vta tiles 2-7
            _in(2304, 3840)                    # tgt tiles 10-21
            _in(VOFF + 8 * (C + 1), NB)        # vta tiles 8-31
            _in(3840, VOFF)                    # tgt tiles 22-31

            # PE warm-up: dummy matmuls with no DMA deps ramp the HAM clock
            # while the input DMAs are in flight.
            def emit_warm(wi):
                warm_ps = psum_st.tile([C, 512], F32, tag="st", bufs=5,
                                       name=f"warm_{wi}")
                nc.tensor.matmul(
                    warm_ps[0:1, :], warm_w, warm_m, start=True, stop=True,
                )

            # ---- attention pipeline ----
            opsum = [
                psum_pv.tile([C, 3, 170], F32, name="opsum_a"),
                psum_pv.tile([C, 3, 170], F32, name="opsum_b"),
                psum_pv.tile([C, 2, 170], F32, name="opsum_c"),
            ]

            st_tiles = {}
            et_tiles = {}

            def emit_st(kt, h):
                # half-tile score matmul: one PSUM bank, ring of 4 so the
                # exp->bank-reuse dependency skips two whole tiles.
                st = psum_st.tile([C, 512], F32, tag="st", bufs=5,
                                  name=f"st_{kt}{'ab'[h]}")
                nc.tensor.matmul(
                    st[:],
                    tgt_ap(kt),
                    qp_ap(h * 512, (h + 1) * 512),
                    start=True, stop=True,
                )
                st_tiles[(kt, h)] = st

            def _et(kt):
                et = et_tiles.get(kt)
                if et is None:
                    et = ets.tile([C, NQ], F16, tag="et", name=f"et_{kt}")
                    et_tiles[kt] = et
                return et

            def emit_exp_act(kt, h):
                # exact exp on Act: et = Exp(st*ln2/1024 + beta)
                et = _et(kt)
                st = st_tiles.pop((kt, h))
                nc.scalar.activation(
                    out=et[:, h * 512 : (h + 1) * 512], in_=st[:], func=AF.Exp,
                    bias=bets_sb[:, 0, kt : kt + 1], scale=LN2_1024,
                )

            def emit_exp_dve(kt, h):
                # f16 exponent-field trick on DVE:
                # et_bits = i16(max(st, CLAMP) + (MAGIC + beta*SC))
                et = _et(kt)
                st = st_tiles.pop((kt, h))
                nc.vector.tensor_scalar(
                    out=et[:, h * 512 : (h + 1) * 512].bitcast(I16), in0=st[:],
                    scalar1=CLAMP, scalar2=bets_sb[:, 1, kt : kt + 1],
                    op0=OP.max, op1=OP.add,
                )

            def emit_pv(kt, chunks=range(QC)):
                et = et_tiles[kt]
                for j in chunks:
                    t, idx = CHUNK_MAP[j]
                    nc.tensor.matmul(
                        opsum[t][:, idx, 0 : C + 1],
                        et[:, j * 128 : (j + 1) * 128],
                        vta_ap(kt),
                        start=(kt == 0 and idx == 0),
                        stop=(kt == KT - 1),
                        skip_group_check=True,
                    )

            def emit_tile(kt):
                emit_st(kt, 0)
                emit_st(kt, 1)
                emit_exp_act(kt, 0)   # low queries: exact exp on Act
                emit_exp_dve(kt, 1)   # high queries: trick exp on DVE

            # ---- software-pipelined emission (PV lags 2 tiles; starting PV
            # early keeps PE's per-tile cost above the exp engines' so the
            # st-ring reuse dependency never stalls PE).
            for wi in range(5):
                emit_warm(wi)
            emit_tile(0)
            emit_tile(1)
            for kt in range(2, KT):
                emit_tile(kt)
                emit_pv(kt - 2)
            emit_pv(KT - 2)
            # last tile: emit grouped by PSUM region so each output region
            # completes (and ships) as early as possible. Copies go on
            # whichever exp engine frees first (DVE exp31b ends first).
            o16 = big.tile([C, QC, C + 1], F16)
            emit_pv(KT - 1, chunks=(0, 1, 2))
            nc.vector.tensor_copy(o16[:, 0:3, :], opsum[0][:, :, 0 : C + 1])
            emit_pv(KT - 1, chunks=(3, 4, 5))
            nc.scalar.activation(
                out=o16[:, 3:6, :], in_=opsum[1][:, :, 0 : C + 1], func=AF.Copy,
            )
            emit_pv(KT - 1, chunks=(6, 7))
            nc.vector.tensor_copy(o16[:, 6:8, :], opsum[2][:, :, 0 : C + 1])
            nc.sync.dma_start(out[:, 0:3, :], o16[:, 0:3, :])
            nc.gpsimd.dma_start(out[:, 3:6, :], o16[:, 3:6, :])
            nc.sync.dma_start(out[:, 6:8, :], o16[:, 6:8, :])

    _split_excess_waits(nc)
    return nc


_NC_CACHE = None


def _get_nc():
    global _NC_CACHE
    if _NC_CACHE is None:
        _NC_CACHE = build_bass()
    return _NC_CACHE


def make_in_maps(source, target, wq, bq, wk, bk, wv, bv, wo, bo):
    source = np.asarray(source, dtype=np.float32).reshape(B, C, N)
    target = np.asarray(target, dtype=np.float32).reshape(B, C, N)
    wq, wk, wv, wo = (np.asarray(x, np.float32) for x in (wq, wk, wv, wo))
    bq, bk, bv, bo = (np.asarray(x, np.float32) for x in (bq, bk, bv, bo))
    scale = np.float32(1.0 / np.sqrt(C))

    M = (wq.T @ wk) * scale                  # [c_src, c_tgt]
    Wvo = wo @ wv                            # [c_out, c_tgt]

    NB = NQ + N + KT * (C + 1)
    VOFF = NQ + N

    qp_b, buf_b, bets_b = [], [], []
    for b in range(B):
        qp_b.append(((M.T @ source[b]) * np.float32(SC)).astype(np.float16))
        buf = np.empty((C, NB), np.float16)
        buf[:, NQ:VOFF] = target[b]
        vp = (Wvo @ target[b]).astype(np.float16)            # [c, N]
        vta = buf[:, VOFF:].reshape(C, KT, C + 1)
        vta[:, :, 0:C] = vp.reshape(C, KT, 128).transpose(2, 1, 0)
        vta[:, :, C] = np.float16(1.0)
        buf_b.append(buf)
        beta = ((bq @ (wk @ target[b])) + np.float32(bq @ bk)) * scale  # [N]
        bvec = beta.reshape(KT, 128).T.astype(np.float32)               # [128, KT]
        bets = np.empty((C, 2, KT), np.float32)
        bets[:, 0, :] = bvec
        bets[:, 1, :] = np.float32(MAGIC) + bvec * np.float32(SC)
        bets_b.append(np.ascontiguousarray(bets))

    in_maps = []
    for core in range(NCORES):
        b, qs = divmod(core, QSHARDS)
        buf = buf_b[b].copy()
        buf[:, 0:NQ] = qp_b[b][:, qs * NQ : (qs + 1) * NQ]
        in_maps.append({
            "buf": buf,
            "bets": bets_b[b],
        })
    return in_maps


def kernel(source, target, wq, bq, wk, bk, wv, bv, wo, bo):
    nc = _get_nc()
    in_maps = make_in_maps(source, target, wq, bq, wk, bk, wv, bv, wo, bo)
    res = run_bass_kernel_spmd(nc, in_maps, core_ids=list(range(NCORES)))
    bvo = (np.asarray(wo, np.float32) @ np.asarray(bv, np.float32)
           + np.asarray(bo, np.float32))                                # [C]
    full = np.empty((B, C, N), dtype=np.float32)
    for core in range(NCORES):
        b, qs = divmod(core, QSHARDS)
        o = np.asarray(res.results[core]["out"], np.float32)            # [p, j, c|d]
        vals = o[:, :, 0:C] / o[:, :, C : C + 1]                        # [p, j, c]
        full[b, :, qs * NQ : (qs + 1) * NQ] = (
            vals.transpose(2, 1, 0).reshape(C, NQ) + bvo[:, None]
        )
    return full.reshape(B, C, D, H, W)
